# revision 1
# baseline (speedup 1.0000x reference)
"""MoE (BruteForceMoELinear) Trainium2 kernel.

Strategy: expert-parallel across 8 NeuronCores. The host (inside
`kernel()`) dispatches token rows by `gate_idx` (stable sort), pads each
expert's token batch to a common capacity C, and hands core e:

  xt  : (128, 4, C)    = x_e^T   laid out [d_inner, d_outer, token]
  w1t : (128, 4, 2048) = W1_e^T  laid out [d_inner, d_outer, f]
  w2t : (128, 16, 512) = W2_e^T  laid out [f_inner, f_outer, d_out]
  sc  : (128, C)       = per-token gate score, replicated over partitions

Each core computes  y_e^T = (W2_e @ relu(W1_e @ x_e^T)) * score  with
float32r matmuls (full-rate fp32 PE path), ReLU fused into the PSUM
eviction on the scalar engine and the gate-score multiply fused into the
second GEMM's PSUM eviction on the vector engine.  The host scatters the
per-expert outputs back to token order and sums the top-k (=2) slots.
"""

import numpy as np

NUM_EXPERT = 8
N_CORES = 8
P = 128

_CACHE = {}


def _build(TN, NCH, KO, FO, repeat=1):
    """Compile the per-core program for capacity C = TN*NCH tokens.

    KO = d_model/128, FO = d_ff/128.  `repeat` re-emits the compute body
    (used only for timing calibration in the dev harness).
    """
    key = (TN, NCH, KO, FO, repeat)
    if key in _CACHE:
        return _CACHE[key]

    import concourse.mybir as mybir
    import concourse.tile as tile
    from concourse import bacc

    f32 = mybir.dt.float32
    f32r = mybir.dt.float32r
    C = TN * NCH
    D_MODEL = KO * P
    D_FF = FO * P

    nc = bacc.Bacc("TRN2", target_bir_lowering=False, debug=False,
                   num_devices=N_CORES)

    xt = nc.dram_tensor("xt", (P, KO, C), f32r, kind="ExternalInput")
    w1t = nc.dram_tensor("w1t", (P, KO, D_FF), f32r, kind="ExternalInput")
    w2t = nc.dram_tensor("w2t", (P, FO, D_MODEL), f32r, kind="ExternalInput")
    sc = nc.dram_tensor("sc", (P, C), f32, kind="ExternalInput")
    yt = nc.dram_tensor("yt", (P, KO, C), f32, kind="ExternalOutput")

    # Holding every chunk's h in SBUF only fits for NCH <= 2; for heavily
    # skewed expert distributions (NCH >= 3) process chunk-major with a
    # rotating 2-buffer h pool instead.
    # NOTE: pools reserve bufs slots PER TAG; the NCH<=2 path uses one
    # persistent tile per chunk tag, so 1 slot per tag suffices (bufs=NCH
    # would double-reserve and overflow SBUF around TN>=400, NCH=2).
    NHB = 1 if NCH <= 2 else 2
    NXB = 1 if NCH <= 2 else 3
    with tile.TileContext(nc) as tc:
        with tc.tile_pool(name="wpool", bufs=1) as wpool, \
             tc.tile_pool(name="xpool", bufs=NXB) as xpool, \
             tc.tile_pool(name="hpool", bufs=NHB) as hpool, \
             tc.tile_pool(name="ypool", bufs=4) as ypool, \
             tc.tile_pool(name="cpool", bufs=1) as cpool, \
             tc.tile_pool(name="ps1", bufs=6, space="PSUM") as ps1, \
             tc.tile_pool(name="ps2", bufs=2, space="PSUM") as ps2:

            bias0 = cpool.tile([P, 1], f32)
            nc.any.memset(bias0[:], 0.0)

            # PE warm-up: dummy matmuls on memset data keep the PE busy
            # through the DMA-priming window so the HAM clock gate is at
            # full rate when the first real matmul issues.
            warm = cpool.tile([P, 64], f32)
            nc.any.memset(warm[:], 0.5)
            wps = ps1.tile([P, 64], f32, name="warm", tag="p1")
            for _i in range(20):
                nc.tensor.matmul(wps[:64, :], warm[:], warm[:],
                                 start=True, stop=True)

            # DMAs execute in emission order on the DMA stream, which is
            # the pacing resource at kernel start.  Emit strictly in
            # consumption order: x(ch0) -> W1 -> x(ch1..) -> W2/sc.
            w1sb = wpool.tile([P, KO, D_FF], f32r)
            w2sb = wpool.tile([P, FO, D_MODEL], f32r)
            scsb = cpool.tile([P, C], f32)
            if NCH <= 2:
                xsbs = [xpool.tile([P, KO, TN], f32r, tag=f"x{ch}",
                                   name=f"xsb{ch}") for ch in range(NCH)]
            else:
                xsbs = None  # allocated per chunk in the fallback loop

            # DMA emission order == consumption order: x/W1 for the first
            # f-block pairwise (fine-grained so the first fo-group starts
            # after ~3 small DMAs), later chunks' x, the rest of W1, then
            # W2 d-blocks and the gate scores.
            FB = 512
            NFB = D_FF // FB
            FPB = FB // P  # fo-groups per W1 f-block
            if NCH <= 2:
                nc.sync.dma_start(w1sb[:, 0:2, 0:FB],
                                  w1t.ap()[:, 0:2, 0:FB])
                nc.sync.dma_start(xsbs[0][:], xt.ap()[:, :, 0:TN])
                nc.sync.dma_start(w1sb[:, 2:KO, 0:FB],
                                  w1t.ap()[:, 2:KO, 0:FB])
                for ch in range(1, NCH):
                    nc.sync.dma_start(xsbs[ch][:],
                                      xt.ap()[:, :, ch * TN:(ch + 1) * TN])
            else:
                nc.sync.dma_start(w1sb[:, :, 0:FB], w1t.ap()[:, :, 0:FB])
            HB = FB // 4
            for hb in range(4, 4 * NFB):
                nc.sync.dma_start(
                    w1sb[:, :, hb * HB:(hb + 1) * HB],
                    w1t.ap()[:, :, hb * HB:(hb + 1) * HB])
            nc.sync.dma_start(w2sb[:, :, 0:P], w2t.ap()[:, :, 0:P])
            nc.sync.dma_start(scsb[:], sc.ap())
            for db in range(1, KO):
                nc.sync.dma_start(w2sb[:, :, db * P:(db + 1) * P],
                                  w2t.ap()[:, :, db * P:(db + 1) * P])

            def gemm1(hsb, xsb, fo):
                p1 = ps1.tile([P, TN], f32, name="p1", tag="p1")
                for ko in range(KO):
                    nc.tensor.matmul(
                        p1[:],
                        w1sb[:, ko, fo * P:(fo + 1) * P],
                        xsb[:, ko, :],
                        start=(ko == 0), stop=(ko == KO - 1))
                nc.scalar.activation(
                    hsb[:, fo, :], p1[:],
                    mybir.ActivationFunctionType.Relu, bias=bias0[:])

            def gemm2(hsb, do, tsl):
                p2 = ps2.tile([P, TN], f32, name="p2", tag="p2")
                for fo in range(FO):
                    nc.tensor.matmul(
                        p2[:],
                        w2sb[:, fo, do * P:(do + 1) * P],
                        hsb[:, fo, :],
                        start=(fo == 0), stop=(fo == FO - 1))
                ysb = ypool.tile([P, TN], f32, tag="y", name="ysb")
                nc.vector.tensor_mul(ysb[:], p2[:], scsb[:, tsl])
                nc.sync.dma_start(yt.ap()[:, do, tsl], ysb[:])

            for _ in range(repeat):
                if NCH <= 2:
                    hsbs = [hpool.tile([P, FO, TN], f32r, tag=f"h{ch}",
                                       name=f"hsb{ch}") for ch in range(NCH)]
                    # phase 1: h = relu(W1 @ x^T); f-block-major so every
                    # W1 block feeds all chunks' matmuls before the next
                    # block is needed (keeps PE ahead of the DMA stream).
                    for fb in range(NFB):
                        for ch in range(NCH):
                            for fo in range(fb * FPB, (fb + 1) * FPB):
                                gemm1(hsbs[ch], xsbs[ch], fo)
                    # phase 2: y^T = (W2 @ h) * score; d-block-major,
                    # streamed out per (db, chunk).
                    for do in range(KO):
                        for ch in range(NCH):
                            gemm2(hsbs[ch], do,
                                  slice(ch * TN, (ch + 1) * TN))
                else:
                    # chunk-major fallback (bounded SBUF for large NCH)
                    for ch in range(NCH):
                        xsb = xpool.tile([P, KO, TN], f32r, tag="x",
                                         name="xsb")
                        nc.sync.dma_start(
                            xsb[:], xt.ap()[:, :, ch * TN:(ch + 1) * TN])
                        hsb = hpool.tile([P, FO, TN], f32r, tag="h",
                                         name="hsb")
                        for fo in range(FO):
                            gemm1(hsb, xsb, fo)
                        for do in range(KO):
                            gemm2(hsb, do, slice(ch * TN, (ch + 1) * TN))

    nc.compile()
    _CACHE[key] = nc
    return nc


def _capacity(max_count):
    """Chunking: NCH chunks of TN tokens; TN in [256, 512] keeps the
    float32r matmul at full rate and within one PSUM bank."""
    maxc = max(int(max_count), 1)
    nch = -(-maxc // 512)
    tn = -(-maxc // (nch * 8)) * 8
    tn = max(tn, 256)
    return tn, nch


_last = {}


def kernel(inp, gate_idx, gate_score, w_htoh4, w_h4toh):
    inp = np.ascontiguousarray(np.asarray(inp, dtype=np.float32))
    gate_idx = np.asarray(gate_idx)
    gate_score = np.asarray(gate_score, dtype=np.float32)
    w_htoh4 = np.asarray(w_htoh4, dtype=np.float32)
    w_h4toh = np.asarray(w_h4toh, dtype=np.float32)

    B, d_model = inp.shape
    n_expert, d_ff, _ = w_htoh4.shape
    assert n_expert == NUM_EXPERT
    KO = d_model // P
    FO = d_ff // P

    gi = gate_idx.astype(np.int64)
    order = np.argsort(gi, kind="stable")
    counts = np.bincount(gi, minlength=NUM_EXPERT)
    idx_split = np.split(order, np.cumsum(counts)[:-1])

    TN, NCH = _capacity(counts.max())
    C = TN * NCH

    # flat per-row gate scores: row 2n+k of inp gets gate_score[n, 0, k]
    scores_flat = gate_score.reshape(-1)

    nc = _build(TN, NCH, KO, FO)

    in_maps = []
    for e in range(NUM_EXPERT):
        idx = idx_split[e]
        cnt = len(idx)
        xT = np.zeros((d_model, C), dtype=np.float32)
        if cnt:
            xT[:, :cnt] = inp[idx].T
        xt_h = np.ascontiguousarray(
            xT.reshape(KO, P, C).transpose(1, 0, 2))
        w1_h = np.ascontiguousarray(
            w_htoh4[e].T.reshape(KO, P, d_ff).transpose(1, 0, 2))
        w2_h = np.ascontiguousarray(
            w_h4toh[e].T.reshape(FO, P, d_model).transpose(1, 0, 2))
        sc_vec = np.zeros((C,), dtype=np.float32)
        if cnt:
            sc_vec[:cnt] = scores_flat[idx]
        sc_h = np.ascontiguousarray(np.broadcast_to(sc_vec, (P, C)))
        in_maps.append({"xt": xt_h, "w1t": w1_h, "w2t": w2_h, "sc": sc_h})

    from concourse import bass_utils
    res = bass_utils.run_bass_kernel_spmd(nc, in_maps,
                                          core_ids=list(range(N_CORES)))

    _last.update(nc=nc, in_maps=in_maps, res=res, TN=TN, NCH=NCH,
                 KO=KO, FO=FO)

    y_full = np.empty((B, d_model), dtype=np.float32)
    for e in range(NUM_EXPERT):
        idx = idx_split[e]
        if len(idx) == 0:
            continue
        yt_h = res.results[e]["yt"]  # (P, KO, C)
        yT = yt_h.transpose(1, 0, 2).reshape(d_model, C)
        y_full[idx] = yT[:, :len(idx)].T

    out = y_full[0::2] + y_full[1::2]
    return np.ascontiguousarray(out, dtype=np.float32)



# revision 8
# speedup vs baseline: 1.0699x; 1.0699x over previous
"""MoE (BruteForceMoELinear) Trainium2 kernel.

Expert-parallel across 8 NeuronCores; host dispatches token rows by
`gate_idx` (stable sort), pads each expert's batch to a common capacity
C = sum(chunks), and hands core e fp16 inputs:

  xt  : (128, KO*C)      x_e^T, gate score pre-folded (relu is
                         positive-homogeneous so s*relu(W1 x) =
                         relu(W1 (s x)) pulls the score through both
                         GEMMs), packed per chunk [ch][ko][tok]
  w1t : (128, FO*KO*128) W1_e^T in fo-major blocks [fo][ko][m]
  w2t : (128, KO*FO*128) W2_e^T in do-major blocks [do][fo][m]

Each core computes y_e^T = W2_e @ relu(W1_e @ x_e^T) with fp16 matmuls
(full-rate PE path, fp32 PSUM accumulate).  Phase 1 runs over two large
token chunks (few, fat ReLU evictions alternating Act/DVE keep PSUM
write-after-read slack); phase 2 re-slices the same fp16 h tiles into
(mid, big, 64) token segments per d-block so the kernel ends on a tiny
chain, whose eviction + single small DMA form the serial tail
(evict -> desc-gen -> copy -> sem -> drain).  Each earlier d-block
ships as ONE row DMA (HWDGE desc-gen is a serial 625ns/DMA resource).
DMA emission order and the phase-1 (fo, chunk) order come from an
analytic model of the DMA launch chain.  The host scatters per-expert
outputs back to token order and sums top-k (=2).
"""

import numpy as np

NUM_EXPERT = 8
N_CORES = 8
P = 128

_CACHE = {}

# cost-model constants used only to pick good static emission orders
_T_GEN0 = 691.0      # first HWDGE desc-gen start
_T_GEN_GAP = 650.0   # SEQ spacing between desc-gen starts
_T_GEN = 625.0       # desc-gen duration
_T_DGE_DELAY = 650.0
_T_SEM = 929.0       # copy-end -> consumable (sem prop + recv)
_BW = 360.0          # DMA bus bytes/ns


def _chunks(maxc):
    """Phase-1 chunking: two near-halves (first ~47%), all <=504 tokens
    (one fp32 PSUM bank); more chunks for very skewed distributions."""
    maxc = max(int(maxc), 1)
    if maxc <= 128:
        return (-(-maxc // 8) * 8,)
    if maxc <= 1008:
        a = int(maxc * 0.47 + 4) // 8 * 8
        b = -(-(maxc - a) // 8) * 8
        return (a, b)
    k = -(-maxc // 504)
    size = -(-maxc // (k * 8)) * 8
    return (size,) * k


def _segments(chunks):
    """Phase-2 token segments (ch, lo, hi), ending with a small tail
    segment carved off the first chunk; y is laid out in this order."""
    if len(chunks) == 1:
        c0 = chunks[0]
        tail = min(64, c0)
        segs = []
        if c0 > tail:
            segs.append((0, 0, c0 - tail))
        segs.append((0, c0 - tail, c0))
        return segs
    tail = 64 if chunks[0] > 64 else max(8, chunks[0] // 2)
    segs = [(0, 0, chunks[0] - tail)]
    segs += [(ch, 0, chunks[ch]) for ch in range(1, len(chunks))]
    segs.append((0, chunks[0] - tail, chunks[0]))
    return segs


def _plan(chunks, KO, FO):
    """DMA emission order + modeled arrival times.

    Each chunk is its own x tile/DMA; W1 streams as fo-pairs.
    Emission: x0, w1b0, x1, w1b1, x2.., w1 rest, w2 d-blocks.
    """
    n_ch = len(chunks)
    w1b = [(f, min(f + 2, FO)) for f in range(0, FO, 2)]
    order = [("x", 0)]
    xi, wi = 1, 0
    while xi < n_ch or wi < len(w1b):
        if wi < len(w1b):
            order.append(("w1",) + w1b[wi])
            wi += 1
        if xi < n_ch:
            order.append(("x", xi))
            xi += 1
    order += [("w2", do) for do in range(KO)]

    x_sem, w1_sem = {}, {}
    bus = 0.0
    for k, ent in enumerate(order):
        gen_end = _T_GEN0 + _T_GEN_GAP * k + _T_GEN
        if ent[0] == "x":
            nb = P * KO * chunks[ent[1]] * 2
        elif ent[0] == "w1":
            nb = P * (ent[2] - ent[1]) * KO * P * 2
        else:
            nb = P * FO * P * 2
        start = max(gen_end + _T_DGE_DELAY, bus)
        bus = start + nb / _BW
        sem = bus + _T_SEM
        if ent[0] == "x":
            x_sem[ent[1]] = sem
        elif ent[0] == "w1":
            for fo in range(ent[1], ent[2]):
                w1_sem[fo] = sem
    return order, x_sem, w1_sem


def _build(chunks, KO, FO, repeat=1):
    """Compile the per-core program for capacity C = sum(chunks)."""
    chunks = tuple(chunks)
    key = (chunks, KO, FO, repeat)
    if key in _CACHE:
        return _CACHE[key]

    import concourse.mybir as mybir
    import concourse.tile as tile
    from concourse import bacc

    f32 = mybir.dt.float32
    f16 = mybir.dt.float16
    C = sum(chunks)
    n_ch = len(chunks)
    xoffs = [sum(chunks[:i]) for i in range(n_ch)]

    order, x_sem, w1_sem = _plan(chunks, KO, FO)
    segs = _segments(chunks)
    yoffs = []
    pos = 0
    for (_, lo, hi) in segs:
        yoffs.append(pos)
        pos += hi - lo
    # phase-1 greedy (fo, ch) order from modeled arrivals
    p1 = [(fo, ch) for fo in range(FO) for ch in range(n_ch)]
    p1.sort(key=lambda p: (max(w1_sem[p[0]], x_sem[p[1]]), p[0], p[1]))

    nc = bacc.Bacc("TRN2", target_bir_lowering=False, debug=False,
                   num_devices=N_CORES)

    xt = nc.dram_tensor("xt", (P, KO * C), f16, kind="ExternalInput")
    w1t = nc.dram_tensor("w1t", (P, FO * KO * P), f16, kind="ExternalInput")
    w2t = nc.dram_tensor("w2t", (P, KO * FO * P), f16, kind="ExternalInput")
    yt = nc.dram_tensor("yt", (P, KO, C), f16, kind="ExternalOutput")

    with tile.TileContext(nc) as tc:
        with tc.tile_pool(name="wpool", bufs=1) as wpool, \
             tc.tile_pool(name="xpool", bufs=1) as xpool, \
             tc.tile_pool(name="hpool", bufs=1) as hpool, \
             tc.tile_pool(name="ypool", bufs=2) as ypool, \
             tc.tile_pool(name="cpool", bufs=1) as cpool, \
             tc.tile_pool(name="ps1", bufs=4, space="PSUM") as ps1, \
             tc.tile_pool(name="ps2", bufs=3, space="PSUM") as ps2, \
             tc.tile_pool(name="psw", bufs=1, space="PSUM") as psw:

            # PE warm-up: fp16 matmuls on memset data start the p-state
            # ramp clock (~3us below 2.4GHz) inside the DMA priming window.
            warm = cpool.tile([P, 512], f16)
            nc.any.memset(warm[:], 0.25)
            wps = psw.tile([P, 512], f32, name="warm", tag="warm")
            for _i in range(6):
                nc.tensor.matmul(wps[:], warm[:, 0:P], warm[:],
                                 start=True, stop=True)

            w1sb = wpool.tile([P, FO * KO * P], f16)
            w2sb = wpool.tile([P, KO * FO * P], f16)
            xsbs = [xpool.tile([P, KO * chunks[ch]], f16, tag=f"x{ch}",
                               name=f"xsb{ch}") for ch in range(n_ch)]

            # input DMAs, single SP HWDGE queue, modeled order
            for ent in order:
                if ent[0] == "x":
                    ch = ent[1]
                    a = KO * xoffs[ch]
                    nc.sync.dma_start(
                        xsbs[ch][:], xt.ap()[:, a:a + KO * chunks[ch]])
                elif ent[0] == "w1":
                    lo, hi = ent[1], ent[2]
                    nc.sync.dma_start(w1sb[:, lo * KO * P:hi * KO * P],
                                      w1t.ap()[:, lo * KO * P:hi * KO * P])
                else:
                    do = ent[1]
                    nc.sync.dma_start(
                        w2sb[:, do * FO * P:(do + 1) * FO * P],
                        w2t.ap()[:, do * FO * P:(do + 1) * FO * P])

            relu = mybir.ActivationFunctionType.Relu

            for _ in range(repeat):
                hsbs = [hpool.tile([P, FO * chunks[ch]], f16, tag=f"h{ch}",
                                   name=f"hsb{ch}") for ch in range(n_ch)]

                # phase 1: h = relu(W1 @ x^T); ReLU eviction alternates
                # Act / DVE to keep either engine off the critical path
                for i, (fo, ch) in enumerate(p1):
                    tn = chunks[ch]
                    p1t = ps1.tile([P, tn], f32, name="p1", tag="p1")
                    for ko in range(KO):
                        nc.tensor.matmul(
                            p1t[:],
                            w1sb[:, (fo * KO + ko) * P:(fo * KO + ko + 1) * P],
                            xsbs[ch][:, ko * tn:(ko + 1) * tn],
                            start=(ko == 0), stop=(ko == KO - 1))
                    hsl = hsbs[ch][:, fo * tn:(fo + 1) * tn]
                    if i % 2 == 0:
                        nc.scalar.activation(hsl, p1t[:], relu)
                    else:
                        nc.vector.tensor_scalar_max(hsl, p1t[:], 0.0)

                # phase 2: y^T = W2 @ h over token segments; evictions
                # alternate DVE/Act (tail segment on DVE) into a per-d-block
                # fp16 staging row; one row DMA per d-block, split on the
                # last d-block so the final serial chain is small.
                for do in range(KO):
                    ysb = ypool.tile([P, C], f16, tag="y", name="ysb")
                    for si, (ch, lo, hi) in enumerate(segs):
                        tn = hi - lo
                        p2t = ps2.tile([P, tn], f32, name="p2", tag="p2")
                        for fo in range(FO):
                            nc.tensor.matmul(
                                p2t[:],
                                w2sb[:, (do * FO + fo) * P:
                                     (do * FO + fo + 1) * P],
                                hsbs[ch][:, fo * chunks[ch] + lo:
                                         fo * chunks[ch] + hi],
                                start=(fo == 0), stop=(fo == FO - 1))
                        ysl = ysb[:, yoffs[si]:yoffs[si] + tn]
                        if si % 2 == 0 or si == len(segs) - 1:
                            nc.vector.tensor_scalar_add(ysl, p2t[:], 0.0)
                        else:
                            nc.scalar.copy(ysl, p2t[:])
                    if do < KO - 1 or len(segs) < 2:
                        nc.sync.dma_start(yt.ap()[:, do, :], ysb[:])
                    else:
                        cut = yoffs[len(segs) - 2]
                        nc.sync.dma_start(yt.ap()[:, do, 0:cut],
                                          ysb[:, 0:cut])
                        nc.sync.dma_start(yt.ap()[:, do, cut:C],
                                          ysb[:, cut:C])

    nc.compile()
    _CACHE[key] = nc
    return nc


_last = {}


def kernel(inp, gate_idx, gate_score, w_htoh4, w_h4toh):
    inp = np.ascontiguousarray(np.asarray(inp, dtype=np.float32))
    gate_idx = np.asarray(gate_idx)
    gate_score = np.asarray(gate_score, dtype=np.float32)
    w_htoh4 = np.asarray(w_htoh4, dtype=np.float32)
    w_h4toh = np.asarray(w_h4toh, dtype=np.float32)

    B, d_model = inp.shape
    n_expert, d_ff, _ = w_htoh4.shape
    assert n_expert == NUM_EXPERT
    KO = d_model // P
    FO = d_ff // P

    gi = gate_idx.astype(np.int64)
    order = np.argsort(gi, kind="stable")
    counts = np.bincount(gi, minlength=NUM_EXPERT)
    idx_split = np.split(order, np.cumsum(counts)[:-1])

    chunks = _chunks(counts.max())
    C = sum(chunks)
    n_ch = len(chunks)
    xoffs = [sum(chunks[:i]) for i in range(n_ch)]
    segs = _segments(chunks)
    yoffs = []
    pos = 0
    for (_, lo, hi) in segs:
        yoffs.append(pos)
        pos += hi - lo

    # fold per-row gate score into x (row 2n+k of inp gets gate_score[n,0,k])
    scores_flat = gate_score.reshape(-1)
    x_scaled = inp * scores_flat[:, None]

    nc = _build(chunks, KO, FO)

    in_maps = []
    for e in range(NUM_EXPERT):
        idx = idx_split[e]
        cnt = len(idx)
        xt_h = np.zeros((P, KO * C), dtype=np.float16)
        for ch, tn in enumerate(chunks):
            a = min(xoffs[ch], cnt)
            b = min(xoffs[ch] + tn, cnt)
            if b <= a:
                continue
            v = b - a
            blk = x_scaled[idx[a:b]].T  # (d_model, v)
            view = xt_h[:, KO * xoffs[ch]:KO * (xoffs[ch] + tn)]
            view.reshape(P, KO, tn)[:, :, :v] = \
                blk.reshape(KO, P, v).transpose(1, 0, 2)
        w1_h = np.ascontiguousarray(
            w_htoh4[e].reshape(FO, P, KO, P).transpose(3, 0, 2, 1)
            .reshape(P, FO * KO * P)).astype(np.float16)
        w2_h = np.ascontiguousarray(
            w_h4toh[e].reshape(KO, P, FO, P).transpose(3, 0, 2, 1)
            .reshape(P, KO * FO * P)).astype(np.float16)
        in_maps.append({"xt": xt_h, "w1t": w1_h, "w2t": w2_h})

    from concourse import bass_utils
    res = bass_utils.run_bass_kernel_spmd(nc, in_maps,
                                          core_ids=list(range(N_CORES)))

    _last.update(nc=nc, in_maps=in_maps, res=res, chunks=chunks,
                 KO=KO, FO=FO)

    y_full = np.empty((B, d_model), dtype=np.float32)
    for e in range(NUM_EXPERT):
        idx = idx_split[e]
        cnt = len(idx)
        if cnt == 0:
            continue
        yt_h = np.asarray(res.results[e]["yt"], dtype=np.float32)  # (P,KO,C)
        yT = yt_h.transpose(1, 0, 2).reshape(d_model, C)
        for si, (ch, lo, hi) in enumerate(segs):
            a = min(xoffs[ch] + lo, cnt)
            b = min(xoffs[ch] + hi, cnt)
            if b <= a:
                continue
            y_full[idx[a:b]] = \
                yT[:, yoffs[si] + (a - xoffs[ch] - lo):
                   yoffs[si] + (b - xoffs[ch] - lo)].T

    out = y_full[0::2] + y_full[1::2]
    return np.ascontiguousarray(out, dtype=np.float32)


# revision 27
# speedup vs baseline: 1.0742x; 1.0040x over previous
"""MoE (BruteForceMoELinear) Trainium2 kernel.

Expert-parallel across 8 NeuronCores; host dispatches token rows by
`gate_idx` (stable sort), pads each expert's batch to a common capacity
C = sum(chunks), and hands core e fp16 inputs:

  xt  : (128, KO*C)      x_e^T, gate score pre-folded (relu is
                         positive-homogeneous so s*relu(W1 x) =
                         relu(W1 (s x)) pulls the score through both
                         GEMMs), packed per chunk [ch][ko][tok]
  w1t : (128, FO*KO*128) W1_e^T in fo-major blocks [fo][ko][m]
  w2t : (128, KO*FO*128) W2_e^T in do-major blocks [do][fo][m]

Each core computes y_e^T = W2_e @ relu(W1_e @ x_e^T) with fp16 matmuls
(full-rate PE path, fp32 PSUM accumulate).  Phase 1 runs over two large
token chunks (few, fat ReLU evictions alternating Act/DVE keep PSUM
write-after-read slack); phase 2 re-slices the same fp16 h tiles into
(mid, big, 64) token segments per d-block so the kernel ends on a tiny
chain, whose eviction + single small DMA form the serial tail
(evict -> desc-gen -> copy -> sem -> drain).  Each earlier d-block
ships as ONE row DMA (HWDGE desc-gen is a serial 625ns/DMA resource).
DMA emission order and the phase-1 (fo, chunk) order come from an
analytic model of the DMA launch chain.  The host scatters per-expert
outputs back to token order and sums top-k (=2).
"""

import numpy as np

NUM_EXPERT = 8
N_CORES = 8
P = 128

_CACHE = {}

# cost-model constants used only to pick good static emission orders
_T_GEN0 = 691.0      # first HWDGE desc-gen start
_T_GEN_GAP = 650.0   # SEQ spacing between desc-gen starts
_T_GEN = 625.0       # desc-gen duration
_T_DGE_DELAY = 650.0
_T_SEM = 929.0       # copy-end -> consumable (sem prop + recv)
_BW = 360.0          # DMA bus bytes/ns


def _chunks(maxc):
    """Phase-1 chunking: two near-halves (first ~47%), all <=504 tokens
    (one fp32 PSUM bank); more chunks for very skewed distributions."""
    maxc = max(int(maxc), 1)
    if maxc <= 128:
        return (-(-maxc // 8) * 8,)
    if maxc <= 1008:
        a = int(maxc * 0.41 + 4) // 8 * 8
        b = -(-(maxc - a) // 8) * 8
        return (a, b)
    k = -(-maxc // 504)
    size = -(-maxc // (k * 8)) * 8
    return (size,) * k


def _segments(chunks):
    """Phase-2 token segments (ch, lo, hi), ending with a small tail
    segment carved off the first chunk; y is laid out in this order."""
    if len(chunks) == 1:
        c0 = chunks[0]
        tail = min(64, c0)
        segs = []
        if c0 > tail:
            segs.append((0, 0, c0 - tail))
        segs.append((0, c0 - tail, c0))
        return segs
    tail = 48 if chunks[0] > 64 else max(8, chunks[0] // 2)
    segs = [(0, 0, chunks[0] - tail)]
    segs += [(ch, 0, chunks[ch]) for ch in range(1, len(chunks))]
    segs.append((0, chunks[0] - tail, chunks[0]))
    return segs


def _plan(chunks, KO, FO):
    """DMA emission order + modeled arrival times.

    Each chunk is its own x tile/DMA; W1 streams as fo-pairs.
    Emission: x0, w1b0, x1, w1b1, x2.., w1 rest, w2 d-blocks.
    """
    n_ch = len(chunks)
    w1b = [(f, min(f + 2, FO)) for f in range(0, FO, 2)]
    order = [("x", 0)]
    xi, wi = 1, 0
    while xi < n_ch or wi < len(w1b):
        if wi < len(w1b):
            order.append(("w1",) + w1b[wi])
            wi += 1
        if xi < n_ch:
            order.append(("x", xi))
            xi += 1
    order += [("w2", do) for do in range(KO)]

    x_sem, w1_sem = {}, {}
    bus = 0.0
    for k, ent in enumerate(order):
        gen_end = _T_GEN0 + _T_GEN_GAP * k + _T_GEN
        if ent[0] == "x":
            nb = P * KO * chunks[ent[1]] * 2
        elif ent[0] == "w1":
            nb = P * (ent[2] - ent[1]) * KO * P * 2
        else:
            nb = P * FO * P * 2
        start = max(gen_end + _T_DGE_DELAY, bus)
        bus = start + nb / _BW
        sem = bus + _T_SEM
        if ent[0] == "x":
            x_sem[ent[1]] = sem
        elif ent[0] == "w1":
            for fo in range(ent[1], ent[2]):
                w1_sem[fo] = sem
    return order, x_sem, w1_sem


def _build(chunks, KO, FO, repeat=1):
    """Compile the per-core program for capacity C = sum(chunks)."""
    chunks = tuple(chunks)
    key = (chunks, KO, FO, repeat)
    if key in _CACHE:
        return _CACHE[key]

    import concourse.mybir as mybir
    import concourse.tile as tile
    from concourse import bacc

    f32 = mybir.dt.float32
    f16 = mybir.dt.float16
    C = sum(chunks)
    n_ch = len(chunks)
    xoffs = [sum(chunks[:i]) for i in range(n_ch)]

    order, x_sem, w1_sem = _plan(chunks, KO, FO)
    segs = _segments(chunks)
    yoffs = []
    pos = 0
    for (_, lo, hi) in segs:
        yoffs.append(pos)
        pos += hi - lo
    # phase-1 greedy (fo, ch) order from modeled arrivals
    p1 = [(fo, ch) for fo in range(FO) for ch in range(n_ch)]
    p1.sort(key=lambda p: (max(w1_sem[p[0]], x_sem[p[1]]), p[0], p[1]))

    nc = bacc.Bacc("TRN2", target_bir_lowering=False, debug=False,
                   num_devices=N_CORES)

    xt = nc.dram_tensor("xt", (P, KO * C), f16, kind="ExternalInput")
    w1t = nc.dram_tensor("w1t", (P, FO * KO * P), f16, kind="ExternalInput")
    w2t = nc.dram_tensor("w2t", (P, KO * FO * P), f16, kind="ExternalInput")
    yt = nc.dram_tensor("yt", (P, KO, C), f16, kind="ExternalOutput")

    with tile.TileContext(nc) as tc:
        with tc.tile_pool(name="wpool", bufs=1) as wpool, \
             tc.tile_pool(name="xpool", bufs=1) as xpool, \
             tc.tile_pool(name="hpool", bufs=1) as hpool, \
             tc.tile_pool(name="ypool", bufs=2) as ypool, \
             tc.tile_pool(name="cpool", bufs=1) as cpool, \
             tc.tile_pool(name="ps1", bufs=4, space="PSUM") as ps1, \
             tc.tile_pool(name="ps2", bufs=3, space="PSUM") as ps2, \
             tc.tile_pool(name="psw", bufs=1, space="PSUM") as psw:

            # PE warm-up: fp16 matmuls on memset data start the p-state
            # ramp clock (~3us below 2.4GHz) inside the DMA priming window.
            warm = cpool.tile([P, 512], f16)
            nc.any.memset(warm[:], 0.25)
            wps = psw.tile([P, 512], f32, name="warm", tag="warm")
            for _i in range(6):
                nc.tensor.matmul(wps[:], warm[:, 0:P], warm[:],
                                 start=True, stop=True)

            w1sb = wpool.tile([P, FO * KO * P], f16)
            w2sb = wpool.tile([P, KO * FO * P], f16)
            xsbs = [xpool.tile([P, KO * chunks[ch]], f16, tag=f"x{ch}",
                               name=f"xsb{ch}") for ch in range(n_ch)]

            # input DMAs, single SP HWDGE queue, modeled order
            for ent in order:
                if ent[0] == "x":
                    ch = ent[1]
                    a = KO * xoffs[ch]
                    nc.sync.dma_start(
                        xsbs[ch][:], xt.ap()[:, a:a + KO * chunks[ch]])
                elif ent[0] == "w1":
                    lo, hi = ent[1], ent[2]
                    nc.sync.dma_start(w1sb[:, lo * KO * P:hi * KO * P],
                                      w1t.ap()[:, lo * KO * P:hi * KO * P])
                else:
                    do = ent[1]
                    nc.sync.dma_start(
                        w2sb[:, do * FO * P:(do + 1) * FO * P],
                        w2t.ap()[:, do * FO * P:(do + 1) * FO * P])

            relu = mybir.ActivationFunctionType.Relu

            for _ in range(repeat):
                hsbs = [hpool.tile([P, FO * chunks[ch]], f16, tag=f"h{ch}",
                                   name=f"hsb{ch}") for ch in range(n_ch)]

                # phase 1: h = relu(W1 @ x^T); ReLU eviction alternates
                # Act / DVE to keep either engine off the critical path
                for i, (fo, ch) in enumerate(p1):
                    tn = chunks[ch]
                    p1t = ps1.tile([P, tn], f32, name="p1", tag="p1")
                    for ko in range(KO):
                        nc.tensor.matmul(
                            p1t[:],
                            w1sb[:, (fo * KO + ko) * P:(fo * KO + ko + 1) * P],
                            xsbs[ch][:, ko * tn:(ko + 1) * tn],
                            start=(ko == 0), stop=(ko == KO - 1))
                    hsl = hsbs[ch][:, fo * tn:(fo + 1) * tn]
                    if i % 2 == 0:
                        nc.scalar.activation(hsl, p1t[:], relu)
                    else:
                        nc.vector.tensor_scalar_max(hsl, p1t[:], 0.0)

                # phase 2: y^T = W2 @ h over token segments; evictions
                # alternate DVE/Act (tail segment on DVE) into a per-d-block
                # fp16 staging row; one row DMA per d-block, split on the
                # last d-block so the final serial chain is small.
                for do in range(KO):
                    last_do = do == KO - 1
                    ysb = ypool.tile([P, C], f16, tag="y", name="ysb")
                    for si, (ch, lo, hi) in enumerate(segs):
                        tn = hi - lo
                        p2t = ps2.tile([P, tn], f32, name="p2", tag="p2")
                        for fo in range(FO):
                            nc.tensor.matmul(
                                p2t[:],
                                w2sb[:, (do * FO + fo) * P:
                                     (do * FO + fo + 1) * P],
                                hsbs[ch][:, fo * chunks[ch] + lo:
                                         fo * chunks[ch] + hi],
                                start=(fo == 0), stop=(fo == FO - 1))
                        ysl = ysb[:, yoffs[si]:yoffs[si] + tn]
                        if si % 2 == 0 or si == len(segs) - 1:
                            nc.vector.tensor_scalar_add(ysl, p2t[:], 0.0)
                        else:
                            nc.scalar.copy(ysl, p2t[:])
                    if not last_do or len(segs) < 2:
                        nc.sync.dma_start(yt.ap()[:, do, :], ysb[:])
                    else:
                        scut = yoffs[len(segs) - 2]
                        nc.sync.dma_start(yt.ap()[:, do, 0:scut],
                                          ysb[:, 0:scut])
                        nc.sync.dma_start(yt.ap()[:, do, scut:C],
                                          ysb[:, scut:C])

    nc.compile()
    _CACHE[key] = nc
    return nc



_last = {}


def kernel(inp, gate_idx, gate_score, w_htoh4, w_h4toh):
    inp = np.ascontiguousarray(np.asarray(inp, dtype=np.float32))
    gate_idx = np.asarray(gate_idx)
    gate_score = np.asarray(gate_score, dtype=np.float32)
    w_htoh4 = np.asarray(w_htoh4, dtype=np.float32)
    w_h4toh = np.asarray(w_h4toh, dtype=np.float32)

    B, d_model = inp.shape
    n_expert, d_ff, _ = w_htoh4.shape
    assert n_expert == NUM_EXPERT
    KO = d_model // P
    FO = d_ff // P

    gi = gate_idx.astype(np.int64)
    order = np.argsort(gi, kind="stable")
    counts = np.bincount(gi, minlength=NUM_EXPERT)
    idx_split = np.split(order, np.cumsum(counts)[:-1])

    chunks = _chunks(counts.max())
    C = sum(chunks)
    n_ch = len(chunks)
    xoffs = [sum(chunks[:i]) for i in range(n_ch)]
    segs = _segments(chunks)
    yoffs = []
    pos = 0
    for (_, lo, hi) in segs:
        yoffs.append(pos)
        pos += hi - lo

    # fold per-row gate score into x (row 2n+k of inp gets gate_score[n,0,k])
    scores_flat = gate_score.reshape(-1)
    x_scaled = inp * scores_flat[:, None]

    nc = _build(chunks, KO, FO)

    in_maps = []
    for e in range(NUM_EXPERT):
        idx = idx_split[e]
        cnt = len(idx)
        xt_h = np.zeros((P, KO * C), dtype=np.float16)
        for ch, tn in enumerate(chunks):
            a = min(xoffs[ch], cnt)
            b = min(xoffs[ch] + tn, cnt)
            if b <= a:
                continue
            v = b - a
            blk = x_scaled[idx[a:b]].T  # (d_model, v)
            view = xt_h[:, KO * xoffs[ch]:KO * (xoffs[ch] + tn)]
            view.reshape(P, KO, tn)[:, :, :v] = \
                blk.reshape(KO, P, v).transpose(1, 0, 2)
        w1_h = np.ascontiguousarray(
            w_htoh4[e].reshape(FO, P, KO, P).transpose(3, 0, 2, 1)
            .reshape(P, FO * KO * P)).astype(np.float16)
        w2_h = np.ascontiguousarray(
            w_h4toh[e].reshape(KO, P, FO, P).transpose(3, 0, 2, 1)
            .reshape(P, KO * FO * P)).astype(np.float16)
        in_maps.append({"xt": xt_h, "w1t": w1_h, "w2t": w2_h})

    from concourse import bass_utils
    res = bass_utils.run_bass_kernel_spmd(nc, in_maps,
                                          core_ids=list(range(N_CORES)))

    _last.update(nc=nc, in_maps=in_maps, res=res, chunks=chunks,
                 KO=KO, FO=FO)

    y_full = np.empty((B, d_model), dtype=np.float32)
    for e in range(NUM_EXPERT):
        idx = idx_split[e]
        cnt = len(idx)
        if cnt == 0:
            continue
        yt_h = np.asarray(res.results[e]["yt"], dtype=np.float32)  # (P,KO,C)
        yT = yt_h.transpose(1, 0, 2).reshape(d_model, C)
        for si, (ch, lo, hi) in enumerate(segs):
            a = min(xoffs[ch] + lo, cnt)
            b = min(xoffs[ch] + hi, cnt)
            if b <= a:
                continue
            y_full[idx[a:b]] = \
                yT[:, yoffs[si] + (a - xoffs[ch] - lo):
                   yoffs[si] + (b - xoffs[ch] - lo)].T

    out = y_full[0::2] + y_full[1::2]
    return np.ascontiguousarray(out, dtype=np.float32)
